# revision 1
# baseline (speedup 1.0000x reference)
"""CPDecoding (embedding_lookup) Trainium2 kernel.

out[n] = sum_c fz[c,n]*fy[c,n]*fx[c,n], where f* is a 1-D linear
interpolation (grid_sample, align_corners=True) of a (96, 512) line table
at per-point coordinates.

Strategy (8 cores, data-parallel over the N=4096*192 points):
  - Host: shard points, pre-permute layouts, pack tables as fp16
    [f0 | delta | pad] rows of 512B (one row per table position i holds
    L[:, i] and L[:, i+1]-L[:, i]).
  - Device: per-point (i0, w) on DVE; per-chunk dma_gather of one 512B row
    per (point, axis) from HBM; DVE interp f0 + w*delta, 3-way product,
    free-axis reduce over the 96 components. ~571us/core (cost model),
    memory-bound-adjacent: 151MB of gathered rows per core.
"""

import numpy as np

N_CORES = 8
N_TOTAL = 4096 * 192
N_CORE = N_TOTAL // N_CORES      # 98304 points per core
P = 128                          # partitions
F = N_CORE // P                  # 768 free blocks (wrapped-128 layout)
GROUPS = 8                       # wrapped-16 partition groups
PHI = N_CORE // 16 // GROUPS     # 768 phi-columns per group
C = 96                           # components
R = 512                          # table resolution
ELEM = 256                       # fp16 elements per table row (512 bytes)
CHUNK_F = 32                     # f-blocks per chunk
CHUNK_PTS = P * CHUNK_F          # 4096 points per chunk
N_CHUNKS = F // CHUNK_F          # 24
CHUNKS_PER_GROUP = N_CHUNKS // GROUPS  # 3
SUBCOLS = CHUNK_PTS // 16        # 256 idx columns per chunk

# axis a -> coordinate column in in_tensor (reference samples [z, y, x] from
# pts[:, 2], pts[:, 1], pts[:, 0])
AXIS_COL = [2, 1, 0]

_BUILT = None


def _build_nc():
    """Build the per-core Bass program (SPMD, identical on all cores)."""
    import concourse.bacc as bacc
    import concourse.bass as bass
    import concourse.tile as tile
    from concourse import mybir
    from concourse.library_config import mlp as lib_mlp

    dt = mybir.dt
    Alu = mybir.AluOpType
    Axis = mybir.AxisListType

    nc = bacc.Bacc("TRN2", target_bir_lowering=False, debug=False,
                   num_devices=N_CORES, num_swdge_queues=1)

    pw128 = nc.dram_tensor("pw128", [P, F * 3], dt.float32,
                           kind="ExternalInput").ap()
    pw16 = nc.dram_tensor("pw16", [P, PHI * 3], dt.float32,
                          kind="ExternalInput").ap()
    tbls = [nc.dram_tensor(f"tbl{a}", [R, ELEM], dt.float16,
                           kind="ExternalInput").ap() for a in range(3)]
    out_d = nc.dram_tensor("out", [P, F], dt.float32,
                           kind="ExternalOutput").ap()

    with tile.TileContext(nc) as tc:
        with tc.tile_pool(name="persist", bufs=1) as pp:
            # persistent tiles
            w_axis = [pp.tile([P, F], dt.float16, tag=f"w{a}",
                              name=f"w_axis{a}") for a in range(3)]
            idx_all = pp.tile([P, 3, PHI], dt.int16, tag="idx")
            out_full = pp.tile([P, F], dt.float32, tag="out")

            # ---------- setup: per-point index/weight math ----------
            with tc.tile_pool(name="setup", bufs=1) as sp:
                s128 = sp.tile([P, F * 3], dt.float32, tag="c0")
                nc.sync.dma_start(s128[:], pw128)
                s16 = sp.tile([P, PHI * 3], dt.float32, tag="c1")
                nc.sync.dma_start(s16[:], pw16)

                def idx_chain(src, n_free, want_w, tg):
                    def tmp(nm, dtype=dt.float32):
                        return sp.tile([P, n_free], dtype,
                                       tag="tmp", bufs=4, name=f"{nm}{tg}")
                    # pos = ((coord + 1) * 0.5) * 511, matching jax op order
                    t1 = tmp("t1")
                    nc.vector.tensor_scalar(t1[:], src[:], 1.0, 0.5,
                                            Alu.add, Alu.mult)
                    pos = tmp("pos")
                    nc.vector.tensor_scalar(pos[:], t1[:], 511.0, None,
                                            Alu.mult)
                    # floor(pos) via int round-trip; the fp->int cast may
                    # truncate or round-to-nearest, so fix up with a compare
                    ii = tmp("ii", dt.int32)
                    nc.vector.tensor_copy(ii[:], pos[:])
                    i0f = tmp("i0f")
                    nc.vector.tensor_copy(i0f[:], ii[:])
                    neg = tmp("neg")
                    nc.vector.tensor_tensor(neg[:], pos[:], i0f[:], Alu.is_lt)
                    i0a = tmp("i0a")
                    nc.vector.tensor_sub(i0a[:], i0f[:], neg[:])
                    i0c = tmp("i0c")
                    nc.vector.tensor_scalar(i0c[:], i0a[:], 510.0, 0.0,
                                            Alu.min, Alu.max)
                    if not want_w:
                        return i0c, None
                    w = tmp("w")
                    nc.vector.tensor_sub(w[:], pos[:], i0c[:])
                    return i0c, w

                _, w128 = idx_chain(s128, F * 3, True, "a")
                w128_3d = w128[:].rearrange("p (f k) -> p f k", k=3)
                for a in range(3):
                    nc.vector.tensor_copy(w_axis[a][:],
                                          w128_3d[:, :, AXIS_COL[a]])

                i0c16, _ = idx_chain(s16, PHI * 3, False, "b")
                i16_3d = i0c16[:].rearrange("p (f k) -> p f k", k=3)
                for a in range(3):
                    nc.vector.tensor_copy(idx_all[:, a, :],
                                          i16_3d[:, :, AXIS_COL[a]])

            # ---------- main loop ----------
            with (
                tc.tile_pool(name="stg", bufs=3) as stg_pool,
                tc.tile_pool(name="gath", bufs=2) as gath_pool,
                tc.tile_pool(name="mid", bufs=2) as mid_pool,
            ):
                with tc.tile_critical():
                    nc.gpsimd.load_library(lib_mlp)
                stg_tiles = {}
                for c in range(N_CHUNKS):
                    grp = c // CHUNKS_PER_GROUP
                    sub = c % CHUNKS_PER_GROUP

                    if sub == 0:
                        # stage group grp's indices, replicated into every
                        # 16-partition band (the SWDGE queue's core pair
                        # reads its own 32-partition window)
                        stg = stg_pool.tile([P, 3, PHI], dt.int16, tag="stg")
                        src = idx_all[16 * grp:16 * (grp + 1), :, :]
                        for b in range(8):
                            nc.sync.dma_start(
                                stg[16 * b:16 * (b + 1), :, :], src)
                        stg_tiles[grp] = stg
                    stg = stg_tiles[grp]

                    az = []
                    for a in range(3):
                        g = gath_pool.tile([P, CHUNK_F, ELEM], dt.float16,
                                           tag=f"g{a}")
                        idxs = stg[:, a, SUBCOLS * sub:SUBCOLS * (sub + 1)]
                        nc.gpsimd.dma_gather(
                            g[:], tbls[a], idxs, CHUNK_PTS, CHUNK_PTS, ELEM,
                            elem_step=ELEM, queue_num=0, single_packet=False)

                        f0 = g[:, :, 0:C]
                        dd = g[:, :, C:2 * C]
                        wb = (w_axis[a][:, CHUNK_F * c:CHUNK_F * (c + 1)]
                              .unsqueeze(2).broadcast_to([P, CHUNK_F, C]))
                        u = mid_pool.tile([P, CHUNK_F, C], dt.float16,
                                          tag="u")
                        nc.vector.tensor_mul(u[:], dd, wb)
                        azt = mid_pool.tile([P, CHUNK_F, C], dt.float16,
                                            tag=f"az{a}")
                        nc.vector.tensor_add(azt[:], f0, u[:])
                        az.append(azt)

                    p = mid_pool.tile([P, CHUNK_F, C], dt.float16, tag="p")
                    nc.vector.tensor_mul(p[:], az[0][:], az[1][:])
                    qq = mid_pool.tile([P, CHUNK_F, C], dt.float16, tag="q")
                    nc.vector.tensor_mul(qq[:], p[:], az[2][:])
                    nc.vector.reduce_sum(
                        out_full[:, CHUNK_F * c:CHUNK_F * (c + 1)],
                        qq[:], axis=Axis.X)

                nc.sync.dma_start(out_d, out_full[:])

    nc.compile()
    return nc


def _host_prep(in_tensor, line_z, line_y, line_x):
    """Build per-core input maps (layout permutations + table packing)."""
    pts = np.ascontiguousarray(in_tensor.reshape(-1, 3).astype(np.float32))

    tables = []
    for L in (line_z, line_y, line_x):
        Lf = np.asarray(L, dtype=np.float32)
        f0 = Lf.T                                    # (512, 96)
        f1 = np.concatenate([Lf.T[1:], Lf.T[-1:]], axis=0)
        row = np.zeros((R, ELEM), dtype=np.float16)
        row[:, 0:C] = f0.astype(np.float16)
        row[:, C:2 * C] = (f1 - f0).astype(np.float16)
        tables.append(row)

    in_maps = []
    for k in range(N_CORES):
        shard = pts[k * N_CORE:(k + 1) * N_CORE]
        pw128 = np.ascontiguousarray(
            shard.reshape(F, P, 3).transpose(1, 0, 2).reshape(P, F * 3))
        pw16 = np.ascontiguousarray(
            shard.reshape(GROUPS, PHI, 16, 3).transpose(0, 2, 1, 3)
            .reshape(P, PHI * 3))
        in_maps.append({
            "pw128": pw128,
            "pw16": pw16,
            "tbl0": tables[0],
            "tbl1": tables[1],
            "tbl2": tables[2],
        })
    return in_maps


def _unshard(results):
    outs = []
    for k in range(N_CORES):
        w = np.asarray(results[k]["out"])            # (128, 768), n = 128f + p
        outs.append(w.T.reshape(-1))
    return np.concatenate(outs).reshape(4096, 192).astype(np.float32)


def kernel(in_tensor, line_z, line_y, line_x):
    global _BUILT
    from concourse.bass_utils import run_bass_kernel_spmd

    if _BUILT is None:
        _BUILT = _build_nc()
    nc = _BUILT
    in_maps = _host_prep(np.asarray(in_tensor), np.asarray(line_z),
                         np.asarray(line_y), np.asarray(line_x))
    res = run_bass_kernel_spmd(nc, in_maps, list(range(N_CORES)))
    return _unshard(res.results)



# revision 2
# speedup vs baseline: 1.2325x; 1.2325x over previous
"""CPDecoding (embedding_lookup) Trainium2 kernel, v3.

out[n] = sum_c fz[c,n]*fy[c,n]*fx[c,n], each f* a 1-D linear interpolation
(grid_sample, align_corners=True) of a (96, 512) line table at per-point
coordinates in [0,1).

Strategy (8 cores, data-parallel over N=4096*192 points):
  - Host: shard points; SORT each shard by z-position so consecutive points
    share z table rows; pack coordinates into gather-layouts; build
      * tblz: coarse z table [512, 256] fp16 rows = [f0(96) | delta(96) |
        row_idx | pad] (512B rows),
      * tbly/tblx: fine pre-interpolated tables [32768, 128] fp16 where row
        j = interp(L, (j+32704)/128) (Q=128 sub-steps, 256B rows).
  - Device: octets of 8 z-sorted points share ONE 512B z-row gather
    (8x descriptor sharing); per-point 256B y/x gathers; exact z interp
    fz = f0 + (posz - row_idx)*delta on DVE; fp16 triple product (one mul
    on the Pool engine); pairwise-tree component reduction.
  - Host: inverse-permute the per-core outputs back to input order.

Quantization error (y/x at Q=128 + fp16): rel err ~5.8e-3 (gate 2e-2).
"""

import numpy as np

N_CORES = 8
N_TOTAL = 4096 * 192
N_CORE = N_TOTAL // N_CORES      # 98304 points per core
P = 128                          # partitions
F = N_CORE // P                  # 768 f-columns
C = 96                           # components
R = 512                          # coarse table resolution
Q = 128                          # fine sub-steps per coarse cell (y/x)
SC = (R - 1) / 2 * Q             # 32704: j = round(coord * SC)
NJ = int(SC) + 1                 # 32705 used fine rows
NJ_PAD = 32768                   # padded fine-table rows
EY = 128                         # y/x gather row elems (fp16) = 256B
EZ = 256                         # z gather row elems (fp16) = 512B
GROUPS = 8                       # staging groups (16-partition bands)
CHUNKS_PER_GROUP = 3
N_CHUNKS = GROUPS * CHUNKS_PER_GROUP          # 24
CHUNK_F = F // N_CHUNKS                       # 32 f-cols per chunk
CHUNK_PTS = P * CHUNK_F                       # 4096 points per chunk
OCT = 8                                       # points per shared z-row
OBLK = CHUNK_F // OCT                         # 4 octet blocks per chunk
OCT_G = CHUNKS_PER_GROUP * CHUNK_PTS // OCT   # 1536 octets per group
# combined int16 idx tile columns: [jy (768) | jx (768) | zi (96)]
JY0, JX0, ZI0 = 0, F, 2 * F
JCOLS = 2 * F + F // OCT         # 1632
# combined fp32 input columns: [zc (768) | y16 (768) | x16 (768) | zo16 (96)]
ZC0, Y0, X0, ZO0 = 0, F, 2 * F, 3 * F
PWCOLS = 3 * F + F // OCT        # 2400

_BUILT = None
_MAPS = None


def _build_static_maps():
    """Static slot->rank index maps (no data dependence).

    Processing slot of chunk c: s in [0,4096) -> [p=s%128, f=32c+s//128].
    Octet grouping: df = s//128 = 8m+u; octet o = m*128+p holds sorted ranks
    r = c*4096 + o*8 + u (u=0..7 consecutive in z-sorted order).
    """
    p = np.arange(P)[:, None]
    f = np.arange(F)[None, :]
    c = f // CHUNK_F
    df = f % CHUNK_F
    m = df // OCT
    u = df % OCT
    rank_pf = c * CHUNK_PTS + (m * P + p) * OCT + u          # [128, 768]

    g = np.arange(GROUPS)[:, None, None]
    t = np.arange(16)[None, :, None]
    phi = np.arange(F)[None, None, :]
    sub = phi // 256
    s = (phi % 256) * 16 + t
    pp = s % P
    dff = s // P
    mm = dff // OCT
    uu = dff % OCT
    rank_y16 = ((3 * g + sub) * CHUNK_PTS + (mm * P + pp) * OCT + uu
                ).reshape(GROUPS * 16, F)                     # [128, 768]

    phio = np.arange(F // OCT)[None, None, :]
    og = phio * 16 + t                                        # octet-in-group
    subo = og // (CHUNK_PTS // OCT)
    rem = og % (CHUNK_PTS // OCT)
    rank_zo = ((3 * g + subo) * CHUNK_PTS + rem * OCT + 3
               ).reshape(GROUPS * 16, F // OCT)               # [128, 96]
    return rank_pf, rank_y16, rank_zo


def _build_nc():
    """Build the per-core Bass program (SPMD, identical on all cores)."""
    import concourse.bacc as bacc
    import concourse.tile as tile
    from concourse import mybir
    from concourse.library_config import mlp as lib_mlp

    dt = mybir.dt
    Alu = mybir.AluOpType
    Axis = mybir.AxisListType

    nc = bacc.Bacc("TRN2", target_bir_lowering=False, debug=False,
                   num_devices=N_CORES, num_swdge_queues=1)

    pwa = nc.dram_tensor("pwa", [P, PWCOLS], dt.float32,
                         kind="ExternalInput").ap()
    tblz = nc.dram_tensor("tblz", [R, EZ], dt.float16,
                          kind="ExternalInput").ap()
    tbly = nc.dram_tensor("tbly", [NJ_PAD, EY], dt.float16,
                          kind="ExternalInput").ap()
    tblx = nc.dram_tensor("tblx", [NJ_PAD, EY], dt.float16,
                          kind="ExternalInput").ap()
    out_d = nc.dram_tensor("out", [P, F], dt.float32,
                           kind="ExternalOutput").ap()

    with tile.TileContext(nc) as tc:
        with tc.tile_pool(name="persist", bufs=1) as pp:
            posz = pp.tile([P, F], dt.float32, tag="posz")
            jall = pp.tile([P, JCOLS], dt.int16, tag="jall")
            out_full = pp.tile([P, F], dt.float32, tag="out")

            # ---------- setup: load coords, index math ----------
            with tc.tile_pool(name="setup", bufs=1) as sp:
                pw = sp.tile([P, PWCOLS], dt.float32, tag="pw")
                nc.sync.dma_start(pw[:], pwa)

                # posz = zc*255.5 + 255.5  (exact coarse position, fp32)
                nc.vector.tensor_scalar(posz[:], pw[:, ZC0:ZC0 + F],
                                        255.5, 255.5, Alu.mult, Alu.add)

                def tmp(nm, ncols, dtype=dt.float32):
                    return sp.tile([P, ncols], dtype, tag="tmp", bufs=6,
                                   name=nm)

                # jy/jx = int16(y*SC + 0.5) (trunc or round both fine)
                for (src0, dstc, nm) in ((Y0, JY0, "jy"), (X0, JX0, "jx")):
                    jf = tmp(nm, F)
                    nc.vector.tensor_scalar(jf[:], pw[:, src0:src0 + F],
                                            float(SC), 0.5, Alu.mult, Alu.add)
                    nc.vector.tensor_copy(jall[:, dstc:dstc + F], jf[:])

                # zi = floor(zo*255.5 + 255.5) with floor fixup, clamp
                nzo = F // OCT
                zposf = tmp("zpos", nzo)
                nc.vector.tensor_scalar(zposf[:], pw[:, ZO0:ZO0 + nzo],
                                        255.5, 255.5, Alu.mult, Alu.add)
                zii = tmp("zii", nzo, dt.int32)
                nc.vector.tensor_copy(zii[:], zposf[:])
                zif = tmp("zif", nzo)
                nc.vector.tensor_copy(zif[:], zii[:])
                zneg = tmp("zneg", nzo)
                nc.vector.tensor_tensor(zneg[:], zposf[:], zif[:], Alu.is_lt)
                zfl = tmp("zfl", nzo)
                nc.vector.tensor_sub(zfl[:], zif[:], zneg[:])
                zcl = tmp("zcl", nzo)
                nc.vector.tensor_scalar(zcl[:], zfl[:], 511.0, 0.0,
                                        Alu.min, Alu.max)
                nc.vector.tensor_copy(jall[:, ZI0:ZI0 + nzo], zcl[:])

            # ---------- main loop ----------
            with (
                tc.tile_pool(name="stg", bufs=2) as stg_pool,
                tc.tile_pool(name="zg", bufs=2) as zg_pool,
                tc.tile_pool(name="gath", bufs=3) as gath_pool,
                tc.tile_pool(name="mid", bufs=3) as mid_pool,
            ):
                with tc.tile_critical():
                    nc.gpsimd.load_library(lib_mlp)

                for g in range(GROUPS):
                    # replicate group g's idx rows into every 16-part band
                    stg = stg_pool.tile([P, JCOLS], dt.int16, tag="stg")
                    src = jall[16 * g:16 * (g + 1), :]
                    for b in range(8):
                        nc.sync.dma_start(stg[16 * b:16 * (b + 1), :], src)

                    # one z-gather per group: 1536 octet rows of 512B
                    zd = zg_pool.tile([P, OCT_G // P, EZ], dt.float16,
                                      tag="zd")
                    nc.gpsimd.dma_gather(
                        zd[:], tblz, stg[:, ZI0:ZI0 + nzo], OCT_G, OCT_G,
                        EZ, elem_step=EZ, queue_num=0, single_packet=False)

                    for sub in range(CHUNKS_PER_GROUP):
                        c = CHUNKS_PER_GROUP * g + sub
                        gath = []
                        for (tb, col0, nm) in ((tbly, JY0, "y"),
                                               (tblx, JX0, "x")):
                            gt = gath_pool.tile([P, CHUNK_F, EY], dt.float16,
                                                tag=f"g{nm}")
                            idxs = stg[:, col0 + 256 * sub:col0 + 256 * (sub + 1)]
                            nc.gpsimd.dma_gather(
                                gt[:], tb, idxs, CHUNK_PTS, CHUNK_PTS, EY,
                                elem_step=EY, queue_num=0, single_packet=False)
                            gath.append(gt)

                        # g2 = fy * fx  (Pool engine to offload DVE)
                        g2 = mid_pool.tile([P, CHUNK_F, C], dt.float16,
                                           tag="g2")
                        nc.gpsimd.tensor_mul(g2[:], gath[0][:, :, 0:C],
                                             gath[1][:, :, 0:C])

                        # wz = posz - row_idx (row idx baked in z-row elem 192)
                        zrow = zd[:, OBLK * sub:OBLK * (sub + 1), :]
                        i0ap = (zrow[:, :, 2 * C:2 * C + 1]
                                .broadcast_to([P, OBLK, OCT]))
                        pz = (posz[:, CHUNK_F * c:CHUNK_F * (c + 1)]
                              .rearrange("p (m u) -> p m u", u=OCT))
                        wz = mid_pool.tile([P, OBLK, OCT], dt.float16,
                                           tag="wz")
                        nc.vector.tensor_sub(wz[:], pz, i0ap)

                        # fz = f0 + wz*delta
                        wzb = wz[:].unsqueeze(3).broadcast_to(
                            [P, OBLK, OCT, C])
                        dzb = (zrow[:, :, C:2 * C].unsqueeze(2)
                               .broadcast_to([P, OBLK, OCT, C]))
                        f0b = (zrow[:, :, 0:C].unsqueeze(2)
                               .broadcast_to([P, OBLK, OCT, C]))
                        u1 = mid_pool.tile([P, CHUNK_F, C], dt.float16,
                                           tag="u1")
                        u1v = u1[:].rearrange("p (m u) e -> p m u e", u=OCT)
                        nc.vector.tensor_mul(u1v, dzb, wzb)
                        fz = mid_pool.tile([P, CHUNK_F, C], dt.float16,
                                           tag="fz")
                        fzv = fz[:].rearrange("p (m u) e -> p m u e", u=OCT)
                        nc.vector.tensor_add(fzv, f0b, u1v)

                        # q = g2 * fz ; tree-reduce 96 -> 12 ; reduce -> out
                        q = mid_pool.tile([P, CHUNK_F, C], dt.float16,
                                          tag="q")
                        nc.vector.tensor_mul(q[:], g2[:], fz[:])
                        t48 = mid_pool.tile([P, CHUNK_F, 48], dt.float16,
                                            tag="t48")
                        nc.vector.tensor_add(t48[:], q[:, :, 0:48],
                                             q[:, :, 48:96])
                        t24 = mid_pool.tile([P, CHUNK_F, 24], dt.float16,
                                            tag="t24")
                        nc.vector.tensor_add(t24[:], t48[:, :, 0:24],
                                             t48[:, :, 24:48])
                        t12 = mid_pool.tile([P, CHUNK_F, 12], dt.float16,
                                            tag="t12")
                        nc.vector.tensor_add(t12[:], t24[:, :, 0:12],
                                             t24[:, :, 12:24])
                        nc.vector.reduce_sum(
                            out_full[:, CHUNK_F * c:CHUNK_F * (c + 1)],
                            t12[:], axis=Axis.X)

                nc.sync.dma_start(out_d, out_full[:])

    nc.compile()
    return nc


def _build_tables(line_z, line_y, line_x):
    Lz = np.asarray(line_z, dtype=np.float32)
    f0 = Lz.T                                     # (512, 96)
    f1 = np.concatenate([Lz.T[1:], Lz.T[-1:]], axis=0)
    tz = np.zeros((R, EZ), dtype=np.float16)
    tz[:, 0:C] = f0.astype(np.float16)
    tz[:, C:2 * C] = (f1 - f0).astype(np.float16)
    tz[:, 2 * C] = np.arange(R, dtype=np.float16)  # row idx, exact in fp16

    fine = []
    j = np.arange(NJ, dtype=np.float64)
    posj = (j + SC) / Q
    i0 = np.clip(np.floor(posj), 0, R - 1).astype(np.int64)
    i1 = np.clip(i0 + 1, 0, R - 1)
    w = (posj - i0).astype(np.float32)[:, None]
    for L in (line_y, line_x):
        Lf = np.asarray(L, dtype=np.float32).T    # (512, 96)
        t = np.zeros((NJ_PAD, EY), dtype=np.float16)
        t[:NJ, 0:C] = (Lf[i0] * (1.0 - w) + Lf[i1] * w).astype(np.float16)
        fine.append(t)
    return tz, fine[0], fine[1]


def _host_prep(in_tensor, line_z, line_y, line_x):
    """Sort/pack per-core inputs; return (in_maps, orders) for unsharding."""
    global _MAPS
    if _MAPS is None:
        _MAPS = _build_static_maps()
    rank_pf, rank_y16, rank_zo = _MAPS

    pts = np.ascontiguousarray(in_tensor.reshape(-1, 3).astype(np.float32))
    tz, ty, tx = _build_tables(line_z, line_y, line_x)

    in_maps, orders = [], []
    for k in range(N_CORES):
        shard = pts[k * N_CORE:(k + 1) * N_CORE]
        order = np.argsort(shard[:, 2], kind="stable")
        srt = shard[order]                         # sorted by z coord
        pw = np.empty((P, PWCOLS), dtype=np.float32)
        pw[:, ZC0:ZC0 + F] = srt[rank_pf, 2]
        pw[:, Y0:Y0 + F] = srt[rank_y16, 1]
        pw[:, X0:X0 + F] = srt[rank_y16, 0]
        pw[:, ZO0:ZO0 + F // OCT] = srt[rank_zo, 2]
        in_maps.append({"pwa": pw, "tblz": tz, "tbly": ty, "tblx": tx})
        orders.append(order)
    return in_maps, orders


def _unshard(results, orders):
    global _MAPS
    rank_pf = _MAPS[0]
    outs = []
    for k in range(N_CORES):
        w = np.asarray(results[k]["out"])          # [128, 768]
        res_sorted = np.empty(N_CORE, dtype=np.float32)
        res_sorted[rank_pf.reshape(-1)] = w.reshape(-1)
        res = np.empty(N_CORE, dtype=np.float32)
        res[orders[k]] = res_sorted
        outs.append(res)
    return np.concatenate(outs).reshape(4096, 192).astype(np.float32)


def kernel(in_tensor, line_z, line_y, line_x):
    global _BUILT
    from concourse.bass_utils import run_bass_kernel_spmd

    if _BUILT is None:
        _BUILT = _build_nc()
    nc = _BUILT
    in_maps, orders = _host_prep(np.asarray(in_tensor), np.asarray(line_z),
                                 np.asarray(line_y), np.asarray(line_x))
    res = run_bass_kernel_spmd(nc, in_maps, list(range(N_CORES)))
    return _unshard(res.results, orders)


# revision 3
# speedup vs baseline: 1.6217x; 1.3158x over previous
"""CPDecoding (embedding_lookup) Trainium2 kernel, v3.

out[n] = sum_c fz[c,n]*fy[c,n]*fx[c,n], each f* a 1-D linear interpolation
(grid_sample, align_corners=True) of a (96, 512) line table at per-point
coordinates in [0,1).

Strategy (8 cores, data-parallel over N=4096*192 points):
  - Host: shard points; SORT each shard by z-position so consecutive points
    share z table rows; pack coordinates into gather-layouts; build
      * tblz: coarse z table [512, 256] fp16 rows = [f0(96) | delta(96) |
        row_idx | pad] (512B rows),
      * tbly/tblx: fine pre-interpolated tables [32768, 128] fp16 where row
        j = interp(L, (j+32704)/128) (Q=128 sub-steps, 256B rows).
  - Device: octets of 8 z-sorted points share ONE 512B z-row gather
    (8x descriptor sharing); per-point 256B y/x gathers; exact z interp
    fz = f0 + (posz - row_idx)*delta on DVE; fp16 triple product (one mul
    on the Pool engine); pairwise-tree component reduction.
  - Host: inverse-permute the per-core outputs back to input order.

Quantization error (y/x at Q=128 + fp16): rel err ~5.8e-3 (gate 2e-2).
"""

import numpy as np

N_CORES = 8
N_TOTAL = 4096 * 192
N_CORE = N_TOTAL // N_CORES      # 98304 points per core
P = 128                          # partitions
F = N_CORE // P                  # 768 f-columns
C = 96                           # components
R = 512                          # coarse table resolution
Q = 128                          # fine sub-steps per coarse cell (y/x)
SC = (R - 1) / 2 * Q             # 32704: j = round(coord * SC)
NJ = int(SC) + 1                 # 32705 used fine rows
NJ_PAD = 32768                   # padded fine-table rows
EY = 128                         # y/x gather row elems (fp16) = 256B
EZ = 256                         # z gather row elems (fp16) = 512B
GROUPS = 8                       # staging groups (16-partition bands)
CHUNKS_PER_GROUP = 3
N_CHUNKS = GROUPS * CHUNKS_PER_GROUP          # 24
CHUNK_F = F // N_CHUNKS                       # 32 f-cols per chunk
CHUNK_PTS = P * CHUNK_F                       # 4096 points per chunk
OCT = 8                                       # points per shared z-row
OBLK = CHUNK_F // OCT                         # 4 octet blocks per chunk
OCT_G = CHUNKS_PER_GROUP * CHUNK_PTS // OCT   # 1536 octets per group
# combined int16 idx tile columns: [jy (768) | jx (768) | zi (96)]
JY0, JX0, ZI0 = 0, F, 2 * F
JCOLS = 2 * F + F // OCT         # 1632
# combined fp32 input columns: [zc (768) | y16 (768) | x16 (768) | zo16 (96)]
ZC0, Y0, X0, ZO0 = 0, F, 2 * F, 3 * F
PWCOLS = 3 * F + F // OCT        # 2400

_BUILT = None
_MAPS = None


def _build_static_maps():
    """Static slot->rank index maps (no data dependence).

    Processing slot of chunk c: s in [0,4096) -> [p=s%128, f=32c+s//128].
    Octet grouping: df = s//128 = 8m+u; octet o = m*128+p holds sorted ranks
    r = c*4096 + o*8 + u (u=0..7 consecutive in z-sorted order).
    """
    p = np.arange(P)[:, None]
    f = np.arange(F)[None, :]
    c = f // CHUNK_F
    df = f % CHUNK_F
    m = df // OCT
    u = df % OCT
    rank_pf = c * CHUNK_PTS + (m * P + p) * OCT + u          # [128, 768]

    g = np.arange(GROUPS)[:, None, None]
    t = np.arange(16)[None, :, None]
    phi = np.arange(F)[None, None, :]
    sub = phi // 256
    s = (phi % 256) * 16 + t
    pp = s % P
    dff = s // P
    mm = dff // OCT
    uu = dff % OCT
    rank_y16 = ((3 * g + sub) * CHUNK_PTS + (mm * P + pp) * OCT + uu
                ).reshape(GROUPS * 16, F)                     # [128, 768]

    phio = np.arange(F // OCT)[None, None, :]
    og = phio * 16 + t                                        # octet-in-group
    subo = og // (CHUNK_PTS // OCT)
    rem = og % (CHUNK_PTS // OCT)
    rank_zo = ((3 * g + subo) * CHUNK_PTS + rem * OCT + 3
               ).reshape(GROUPS * 16, F // OCT)               # [128, 96]
    return rank_pf, rank_y16, rank_zo


def _build_nc():
    """Build the per-core Bass program (SPMD, identical on all cores)."""
    import concourse.bacc as bacc
    import concourse.tile as tile
    from concourse import mybir
    from concourse.library_config import mlp as lib_mlp

    dt = mybir.dt
    Alu = mybir.AluOpType
    Axis = mybir.AxisListType

    nc = bacc.Bacc("TRN2", target_bir_lowering=False, debug=False,
                   num_devices=N_CORES, num_swdge_queues=1)

    pwa = nc.dram_tensor("pwa", [P, PWCOLS], dt.float32,
                         kind="ExternalInput").ap()
    tblz = nc.dram_tensor("tblz", [R, EZ], dt.float16,
                          kind="ExternalInput").ap()
    tbly = nc.dram_tensor("tbly", [NJ_PAD, EY], dt.float16,
                          kind="ExternalInput").ap()
    tblx = nc.dram_tensor("tblx", [NJ_PAD, EY], dt.float16,
                          kind="ExternalInput").ap()
    out_d = nc.dram_tensor("out", [P, F], dt.float32,
                           kind="ExternalOutput").ap()

    with tile.TileContext(nc) as tc:
        with tc.tile_pool(name="persist", bufs=1) as pp:
            posz = pp.tile([P, F], dt.float32, tag="posz")
            jall = pp.tile([P, JCOLS], dt.int16, tag="jall")
            out_full = pp.tile([P, F], dt.float32, tag="out")

            # ---------- setup: load coords, index math ----------
            with tc.tile_pool(name="setup", bufs=1) as sp:
                pw = sp.tile([P, PWCOLS], dt.float32, tag="pw")
                nc.sync.dma_start(pw[:], pwa)

                # posz = zc*255.5 + 255.5  (exact coarse position, fp32)
                nc.vector.tensor_scalar(posz[:], pw[:, ZC0:ZC0 + F],
                                        255.5, 255.5, Alu.mult, Alu.add)

                def tmp(nm, ncols, dtype=dt.float32):
                    return sp.tile([P, ncols], dtype, tag="tmp", bufs=6,
                                   name=nm)

                # jy/jx = int16(y*SC + 0.5) (trunc or round both fine)
                for (src0, dstc, nm) in ((Y0, JY0, "jy"), (X0, JX0, "jx")):
                    jf = tmp(nm, F)
                    nc.vector.tensor_scalar(jf[:], pw[:, src0:src0 + F],
                                            float(SC), 0.5, Alu.mult, Alu.add)
                    nc.vector.tensor_copy(jall[:, dstc:dstc + F], jf[:])

                # zi = floor(zo*255.5 + 255.5) with floor fixup, clamp
                nzo = F // OCT
                zposf = tmp("zpos", nzo)
                nc.vector.tensor_scalar(zposf[:], pw[:, ZO0:ZO0 + nzo],
                                        255.5, 255.5, Alu.mult, Alu.add)
                zii = tmp("zii", nzo, dt.int32)
                nc.vector.tensor_copy(zii[:], zposf[:])
                zif = tmp("zif", nzo)
                nc.vector.tensor_copy(zif[:], zii[:])
                zneg = tmp("zneg", nzo)
                nc.vector.tensor_tensor(zneg[:], zposf[:], zif[:], Alu.is_lt)
                zfl = tmp("zfl", nzo)
                nc.vector.tensor_sub(zfl[:], zif[:], zneg[:])
                zcl = tmp("zcl", nzo)
                nc.vector.tensor_scalar(zcl[:], zfl[:], 511.0, 0.0,
                                        Alu.min, Alu.max)
                nc.vector.tensor_copy(jall[:, ZI0:ZI0 + nzo], zcl[:])

            # ---------- main loop ----------
            with (
                tc.tile_pool(name="stg", bufs=2) as stg_pool,
                tc.tile_pool(name="zg", bufs=2) as zg_pool,
                tc.tile_pool(name="gath", bufs=3) as gath_pool,
                tc.tile_pool(name="mid", bufs=3) as mid_pool,
            ):
                with tc.tile_critical():
                    nc.gpsimd.load_library(lib_mlp)

                for g in range(GROUPS):
                    # replicate group g's idx rows into every 16-part band
                    stg = stg_pool.tile([P, JCOLS], dt.int16, tag="stg")
                    src = jall[16 * g:16 * (g + 1), :]
                    for b in range(8):
                        nc.sync.dma_start(stg[16 * b:16 * (b + 1), :], src)

                    # one z-gather per group: 1536 octet rows of 512B
                    zd = zg_pool.tile([P, OCT_G // P, EZ], dt.float16,
                                      tag="zd")
                    nc.gpsimd.dma_gather(
                        zd[:], tblz, stg[:, ZI0:ZI0 + nzo], OCT_G, OCT_G,
                        EZ, elem_step=EZ, queue_num=0, single_packet=False)

                    for sub in range(CHUNKS_PER_GROUP):
                        c = CHUNKS_PER_GROUP * g + sub
                        gath = []
                        for (tb, col0, nm) in ((tbly, JY0, "y"),
                                               (tblx, JX0, "x")):
                            gt = gath_pool.tile([P, CHUNK_F, EY], dt.float16,
                                                tag=f"g{nm}")
                            idxs = stg[:, col0 + 256 * sub:col0 + 256 * (sub + 1)]
                            nc.gpsimd.dma_gather(
                                gt[:], tb, idxs, CHUNK_PTS, CHUNK_PTS, EY,
                                elem_step=EY, queue_num=0, single_packet=False)
                            gath.append(gt)

                        # g2 = fy * fx
                        g2 = mid_pool.tile([P, CHUNK_F, C], dt.float16,
                                           tag="g2")
                        nc.vector.tensor_mul(g2[:], gath[0][:, :, 0:C],
                                             gath[1][:, :, 0:C])

                        # wz = posz - row_idx (row idx baked in z-row elem 192)
                        zrow = zd[:, OBLK * sub:OBLK * (sub + 1), :]
                        i0ap = (zrow[:, :, 2 * C:2 * C + 1]
                                .broadcast_to([P, OBLK, OCT]))
                        pz = (posz[:, CHUNK_F * c:CHUNK_F * (c + 1)]
                              .rearrange("p (m u) -> p m u", u=OCT))
                        wz = mid_pool.tile([P, OBLK, OCT], dt.float16,
                                           tag="wz")
                        nc.vector.tensor_sub(wz[:], pz, i0ap)

                        # fz = f0 + wz*delta
                        wzb = wz[:].unsqueeze(3).broadcast_to(
                            [P, OBLK, OCT, C])
                        dzb = (zrow[:, :, C:2 * C].unsqueeze(2)
                               .broadcast_to([P, OBLK, OCT, C]))
                        f0b = (zrow[:, :, 0:C].unsqueeze(2)
                               .broadcast_to([P, OBLK, OCT, C]))
                        u1 = mid_pool.tile([P, CHUNK_F, C], dt.float16,
                                           tag="u1")
                        u1v = u1[:].rearrange("p (m u) e -> p m u e", u=OCT)
                        nc.vector.tensor_mul(u1v, dzb, wzb)
                        fz = mid_pool.tile([P, CHUNK_F, C], dt.float16,
                                           tag="fz")
                        fzv = fz[:].rearrange("p (m u) e -> p m u e", u=OCT)
                        nc.vector.tensor_add(fzv, f0b, u1v)

                        # q = g2 * fz ; tree-reduce 96 -> 12 ; reduce -> out
                        q = mid_pool.tile([P, CHUNK_F, C], dt.float16,
                                          tag="q")
                        nc.vector.tensor_mul(q[:], g2[:], fz[:])
                        t48 = mid_pool.tile([P, CHUNK_F, 48], dt.float16,
                                            tag="t48")
                        nc.vector.tensor_add(t48[:], q[:, :, 0:48],
                                             q[:, :, 48:96])
                        t24 = mid_pool.tile([P, CHUNK_F, 24], dt.float16,
                                            tag="t24")
                        nc.vector.tensor_add(t24[:], t48[:, :, 0:24],
                                             t48[:, :, 24:48])
                        t12 = mid_pool.tile([P, CHUNK_F, 12], dt.float16,
                                            tag="t12")
                        nc.vector.tensor_add(t12[:], t24[:, :, 0:12],
                                             t24[:, :, 12:24])
                        nc.vector.reduce_sum(
                            out_full[:, CHUNK_F * c:CHUNK_F * (c + 1)],
                            t12[:], axis=Axis.X)

                nc.sync.dma_start(out_d, out_full[:])

    nc.compile()
    return nc


def _build_tables(line_z, line_y, line_x):
    Lz = np.asarray(line_z, dtype=np.float32)
    f0 = Lz.T                                     # (512, 96)
    f1 = np.concatenate([Lz.T[1:], Lz.T[-1:]], axis=0)
    tz = np.zeros((R, EZ), dtype=np.float16)
    tz[:, 0:C] = f0.astype(np.float16)
    tz[:, C:2 * C] = (f1 - f0).astype(np.float16)
    tz[:, 2 * C] = np.arange(R, dtype=np.float16)  # row idx, exact in fp16

    fine = []
    j = np.arange(NJ, dtype=np.float64)
    posj = (j + SC) / Q
    i0 = np.clip(np.floor(posj), 0, R - 1).astype(np.int64)
    i1 = np.clip(i0 + 1, 0, R - 1)
    w = (posj - i0).astype(np.float32)[:, None]
    for L in (line_y, line_x):
        Lf = np.asarray(L, dtype=np.float32).T    # (512, 96)
        t = np.zeros((NJ_PAD, EY), dtype=np.float16)
        t[:NJ, 0:C] = (Lf[i0] * (1.0 - w) + Lf[i1] * w).astype(np.float16)
        fine.append(t)
    return tz, fine[0], fine[1]


def _host_prep(in_tensor, line_z, line_y, line_x):
    """Sort/pack per-core inputs; return (in_maps, orders) for unsharding."""
    global _MAPS
    if _MAPS is None:
        _MAPS = _build_static_maps()
    rank_pf, rank_y16, rank_zo = _MAPS

    pts = np.ascontiguousarray(in_tensor.reshape(-1, 3).astype(np.float32))
    tz, ty, tx = _build_tables(line_z, line_y, line_x)

    in_maps, orders = [], []
    for k in range(N_CORES):
        shard = pts[k * N_CORE:(k + 1) * N_CORE]
        order = np.argsort(shard[:, 2], kind="stable")
        srt = shard[order]                         # sorted by z coord
        pw = np.empty((P, PWCOLS), dtype=np.float32)
        pw[:, ZC0:ZC0 + F] = srt[rank_pf, 2]
        pw[:, Y0:Y0 + F] = srt[rank_y16, 1]
        pw[:, X0:X0 + F] = srt[rank_y16, 0]
        pw[:, ZO0:ZO0 + F // OCT] = srt[rank_zo, 2]
        in_maps.append({"pwa": pw, "tblz": tz, "tbly": ty, "tblx": tx})
        orders.append(order)
    return in_maps, orders


def _unshard(results, orders):
    global _MAPS
    rank_pf = _MAPS[0]
    outs = []
    for k in range(N_CORES):
        w = np.asarray(results[k]["out"])          # [128, 768]
        res_sorted = np.empty(N_CORE, dtype=np.float32)
        res_sorted[rank_pf.reshape(-1)] = w.reshape(-1)
        res = np.empty(N_CORE, dtype=np.float32)
        res[orders[k]] = res_sorted
        outs.append(res)
    return np.concatenate(outs).reshape(4096, 192).astype(np.float32)


def kernel(in_tensor, line_z, line_y, line_x):
    global _BUILT
    from concourse.bass_utils import run_bass_kernel_spmd

    if _BUILT is None:
        _BUILT = _build_nc()
    nc = _BUILT
    in_maps, orders = _host_prep(np.asarray(in_tensor), np.asarray(line_z),
                                 np.asarray(line_y), np.asarray(line_x))
    res = run_bass_kernel_spmd(nc, in_maps, list(range(N_CORES)))
    return _unshard(res.results, orders)
